# revision 20
# baseline (speedup 1.0000x reference)
"""MoE feed-forward (8 experts, top-2, SwiGLU) on 8 Trainium2 NeuronCores.

Strategy: routed expert parallelism with pair balancing. Experts are paired
largest-count-with-smallest; the two cores of a pair each hold BOTH experts'
weights (bf16 halves them to ~17MB, fitting SBUF) and each processes half of
both experts' routed tokens, so per-core work tracks the pair mean instead
of the max expert. Host gathers each core's tokens compactly (feature-major),
computes the router top-2 softmax exactly, and scatter-adds the compact
per-core outputs into the full [B, S, D] result.

Compute is bf16 (fp32 PSUM accumulate): the PE streams 1 column/cycle for
both fp32r and bf16, but bf16 halves DMA bytes and, via fast-weight-load,
halves LDWEIGHTS so the matmul stream — not the weight loads — is the
limiter. All device inputs are host-packed so every DMA is one contiguous
run per partition (descriptor generation is ~4ns/descriptor on the issuing
engine and would otherwise serialize the head). Inputs stream on two HWDGE
queues (weights on sync, activations on scalar), outputs on the gpsimd
queue so no input queue's end-of-kernel drain waits on them. A few
memset-fed warmup matmuls fill the PE queue during the DMA-limited head.
Measured end-to-end rel-err ~4e-3 (gate: 2e-2).
"""

import os
import sys
import time

sys.path.insert(0, "/opt/trn_rl_repo")

import numpy as np

# ---------------------------------------------------------------------------
# Problem constants (hardcoded per contract)
B, S, D, E, I, TOPK = 2, 2048, 1024, 8, 1408, 2
T = B * S  # 4096 tokens
P = 128
D_T = D // P   # 8 d-tiles
I_T = I // P   # 11 i-tiles
N_CORES = 8
N_WARMUP = 7   # dummy MMs filling the pre-supply window (PE idle anyway)
# i-tile blocks for the wg/wu weight streams (pipelined arrival)
WBLOCKS = [(0, 2), (2, 5), (5, 8), (8, 11)]
WSZ = D_T * I   # one weight set's wg/wu columns per partition
DSZ = I_T * D   # one weight set's wd columns per partition

_VERBOSE = bool(int(os.environ.get("KERNEL_VERBOSE", "0")))
_PAIR = bool(int(os.environ.get("KERNEL_PAIR", "0")))


def _log(msg):
    if _VERBOSE:
        print(f"[kernel] {msg}", flush=True)


def host_gating(x2d: np.ndarray, gate_w: np.ndarray):
    """Exact router: scores -> top-2 -> softmax. Returns gating [T, E] fp32."""
    scores = x2d.astype(np.float64) @ gate_w.astype(np.float64).T  # [T, E]
    idx = np.argsort(-scores, axis=-1, kind="stable")[:, :TOPK]  # [T, 2]
    top = np.take_along_axis(scores, idx, axis=-1)  # [T, 2] descending
    m = top[:, :1]
    ex = np.exp(top - m)
    probs = ex / ex.sum(axis=-1, keepdims=True)  # [T, 2]
    gating = np.zeros((x2d.shape[0], E), dtype=np.float64)
    np.put_along_axis(gating, idx, probs, axis=-1)
    return gating.astype(np.float32)


def pack_rows(a3: np.ndarray) -> np.ndarray:
    """[G, P, W] -> [P, G*W]: per-partition-contiguous device layout."""
    return np.ascontiguousarray(a3.transpose(1, 0, 2).reshape(P, -1))


# ---------------------------------------------------------------------------
# Bass kernel builder


def build_nc_routed(chunk_plan, n_wsets, n_cores=N_CORES):
    """Generalized routed builder. chunk_plan: tuple of (tc, wset) per chunk;
    each core computes SwiGLU for its host-gathered token columns, chunk ci
    using weight set chunk_plan[ci][1], scales by the gating prob, and
    writes the compact [D, cap] output."""
    import concourse.mybir as mybir
    import concourse.tile as tile
    from concourse import bacc

    f32 = mybir.dt.float32
    bf16 = mybir.dt.bfloat16
    cap = sum(tc for tc, _ in chunk_plan)
    tc_max = max(tc for tc, _ in chunk_plan)
    n_chunks = len(chunk_plan)

    nc = bacc.Bacc("TRN2", debug=False, num_devices=n_cores)

    xc_d = nc.dram_tensor("xc", [P, D_T * cap], bf16, kind="ExternalInput")
    wg_d = nc.dram_tensor("wg", [P, n_wsets * WSZ], bf16, kind="ExternalInput")
    wu_d = nc.dram_tensor("wu", [P, n_wsets * WSZ], bf16, kind="ExternalInput")
    wd_d = nc.dram_tensor("wd", [P, n_wsets * DSZ], bf16, kind="ExternalInput")
    gprob_d = nc.dram_tensor("gprob", [1, cap], bf16, kind="ExternalInput")
    ycomp_d = nc.dram_tensor("ycomp", [D, cap], bf16, kind="ExternalOutput")
    ycomp_r = ycomp_d.ap().rearrange("(do dp) t -> dp do t", dp=P)

    # flat offsets for the block-major wg/wu layout
    blk_of_tile = {}
    blk_off = {}
    off = 0
    for b, (s0, s1) in enumerate(WBLOCKS):
        blk_off[b] = off
        for i_o in range(s0, s1):
            blk_of_tile[i_o] = (b, i_o - s0)
        off += D_T * (s1 - s0) * P
    assert off == WSZ

    def w_slice(w_sb, ws, d_o, i_o):
        b, j = blk_of_tile[i_o]
        s0, s1 = WBLOCKS[b]
        bw = (s1 - s0) * P
        o = ws * WSZ + blk_off[b] + d_o * bw + j * P
        return w_sb[:, o:o + P]

    def blk_rng(b):
        s0, s1 = WBLOCKS[b]
        return blk_off[b], blk_off[b] + D_T * (s1 - s0) * P

    # chunk column offsets
    xoff, toff = [], []
    xo = to = 0
    for tc, _ in chunk_plan:
        xoff.append(xo)
        toff.append(to)
        xo += D_T * tc
        to += tc

    with tile.TileContext(nc) as tcx:
        with tcx.tile_pool(name="wpool", bufs=1) as wpool, \
             tcx.tile_pool(name="xpool", bufs=n_chunks) as xpool, \
             tcx.tile_pool(name="hpool", bufs=2) as hpool, \
             tcx.tile_pool(name="ypool", bufs=2) as ypool, \
             tcx.tile_pool(name="gspool", bufs=3) as gspool, \
             tcx.tile_pool(name="gbpool", bufs=2) as gbpool, \
             tcx.tile_pool(name="psg", bufs=2, space="PSUM") as psg, \
             tcx.tile_pool(name="psu", bufs=2, space="PSUM") as psu, \
             tcx.tile_pool(name="psy", bufs=2, space="PSUM") as psy, \
             tcx.tile_pool(name="psb", bufs=1, space="PSUM") as psb:

            # ---- consts: memset, no DMA ----
            warm_c = wpool.tile([1, 512], bf16)
            nc.gpsimd.memset(warm_c[:], 1.0)

            # PE queue filler for the DMA-limited head
            warm_ps = psb.tile([P, 512], f32, tag="gbps", name="warm")
            for _ in range(N_WARMUP):
                nc.tensor.matmul(warm_ps[:], warm_c[:, :P], warm_c[:],
                                 start=True, stop=True)

            wg_sb = wpool.tile([P, n_wsets * WSZ], bf16)
            wu_sb = wpool.tile([P, n_wsets * WSZ], bf16)
            wd_sb = wpool.tile([P, n_wsets * DSZ], bf16)
            gprob_sb = wpool.tile([1, cap], bf16)

            xts = [xpool.tile([P, D_T * tc], bf16, tag="xt", name=f"xt{ci}")
                   for ci, (tc, _) in enumerate(chunk_plan)]

            # activations on the scalar HWDGE queue; weights stream on the
            # sync queue in consumption order (wg/wu interleaved by block)
            tc0 = chunk_plan[0][0]
            for s in range(0, D_T, 2):  # chunk 0 in d_o pairs
                nc.scalar.dma_start(xts[0][:, s * tc0:(s + 2) * tc0],
                                    xc_d.ap()[:, s * tc0:(s + 2) * tc0])
            nc.scalar.dma_start(gprob_sb[:], gprob_d.ap())
            for ci in range(1, n_chunks):
                o = xoff[ci]
                nc.scalar.dma_start(
                    xts[ci][:], xc_d.ap()[:, o:o + D_T * chunk_plan[ci][0]])

            for b in range(len(WBLOCKS)):
                o0, o1 = blk_rng(b)
                nc.sync.dma_start(wg_sb[:, o0:o1], wg_d.ap()[:, o0:o1])
                nc.sync.dma_start(wu_sb[:, o0:o1], wu_d.ap()[:, o0:o1])
            nc.sync.dma_start(wd_sb[:, :DSZ], wd_d.ap()[:, :DSZ])
            if n_wsets > 1:  # weight set 1 streams behind set 0's compute
                nc.sync.dma_start(wg_sb[:, WSZ:], wg_d.ap()[:, WSZ:])
                nc.sync.dma_start(wu_sb[:, WSZ:], wu_d.ap()[:, WSZ:])
                nc.sync.dma_start(wd_sb[:, DSZ:], wd_d.ap()[:, DSZ:])

            for ci, (tc, ws) in enumerate(chunk_plan):
                t0 = toff[ci]
                xt = xts[ci]

                h = hpool.tile([P, I_T, tc_max], bf16, tag="h")
                for i_o in range(I_T):
                    pg = psg.tile([P, tc_max], f32, tag="pg")
                    pu = psu.tile([P, tc_max], f32, tag="pu")
                    for d_o in range(D_T):
                        nc.tensor.matmul(
                            pg[:, :tc], w_slice(wg_sb, ws, d_o, i_o),
                            xt[:, d_o * tc:(d_o + 1) * tc],
                            start=(d_o == 0), stop=(d_o == D_T - 1))
                    for d_o in range(D_T):
                        nc.tensor.matmul(
                            pu[:, :tc], w_slice(wu_sb, ws, d_o, i_o),
                            xt[:, d_o * tc:(d_o + 1) * tc],
                            start=(d_o == 0), stop=(d_o == D_T - 1))
                    gs = gspool.tile([P, tc_max], bf16, tag="gs")
                    nc.scalar.activation(gs[:, :tc], pg[:, :tc],
                                         mybir.ActivationFunctionType.Silu)
                    nc.vector.tensor_mul(out=h[:, i_o, :tc], in0=gs[:, :tc],
                                         in1=pu[:, :tc])

                # broadcast this chunk's gating row to 128 partitions just
                # before the down-proj that consumes it
                gb_ps = psb.tile([P, tc_max], f32, tag="gbps")
                nc.tensor.matmul(gb_ps[:, :tc], warm_c[:, :P],
                                 gprob_sb[:, t0:t0 + tc],
                                 start=True, stop=True)
                gb_sb = gbpool.tile([P, tc_max], f32, tag="gb")
                nc.vector.tensor_copy(out=gb_sb[:, :tc], in_=gb_ps[:, :tc])

                yout = ypool.tile([P, D_T, tc_max], bf16, tag="yout")
                for d_o in range(D_T):
                    py = psy.tile([P, tc_max], f32, tag="py")
                    for i_o in range(I_T):
                        nc.tensor.matmul(
                            py[:, :tc],
                            wd_sb[:, ws * DSZ + i_o * D + d_o * P:
                                  ws * DSZ + i_o * D + d_o * P + P],
                            h[:, i_o, :tc],
                            start=(i_o == 0), stop=(i_o == I_T - 1))
                    nc.vector.tensor_mul(out=yout[:, d_o, :tc],
                                         in0=py[:, :tc], in1=gb_sb[:, :tc])
                # outputs on the (otherwise idle) gpsimd queue; last chunk
                # per-d_o so the final transfer after the last matmul is small
                ostep = 1 if ci == n_chunks - 1 else 2
                for s0 in range(0, D_T, ostep):
                    nc.gpsimd.dma_start(
                        ycomp_r[:, s0:s0 + ostep, t0:t0 + tc],
                        yout[:, s0:s0 + ostep, :tc])

    nc.compile()
    return nc


# ---------------------------------------------------------------------------
# Host-side wrapper

_CACHED = {}


def _get_nc_routed(chunk_plan, n_wsets, n_cores=N_CORES):
    key = (chunk_plan, n_wsets, n_cores)
    if key not in _CACHED:
        t0 = time.time()
        _CACHED[key] = build_nc_routed(chunk_plan, n_wsets, n_cores)
        _log(f"built bass program (chunks={chunk_plan}, wsets={n_wsets}) "
             f"in {time.time() - t0:.1f}s")
    return _CACHED[key]


def _round_up(v, m):
    return (v + m - 1) // m * m


def _slot_chunks(s):
    """Chunk widths covering s columns: fewest <=512 chunks, evenly sized."""
    n = max(1, -(-s // 512))
    tc = -(-s // n)
    return [tc] * n


def _pack_w(w, bf16, blocks=None):
    """w: [G*P, W] source -> device flat [P, G*W] (optionally block-major)."""
    a3 = w.reshape(-1, P, w.shape[-1])  # [G, P, W]
    if blocks is None:
        return pack_rows(a3).astype(bf16)
    parts = [pack_rows(a3[:, :, s0 * P:s1 * P]) for s0, s1 in blocks]
    return np.concatenate(parts, axis=1).astype(bf16)


def make_in_maps_routed(x, gate_w, gate_proj_w, up_proj_w, down_proj_w):
    """Returns (in_maps, seglists, chunk_plan, n_wsets)."""
    from concurrent.futures import ThreadPoolExecutor
    import ml_dtypes

    bf16 = ml_dtypes.bfloat16
    x2d = np.ascontiguousarray(np.asarray(x, np.float32).reshape(T, D))
    gating = host_gating(x2d, np.asarray(gate_w, np.float32))  # [T, E]
    idx_list = [np.nonzero(gating[:, c] > 0)[0].astype(np.int64)
                for c in range(N_CORES)]
    n_list = [len(ix) for ix in idx_list]
    xT_bf = np.ascontiguousarray(x2d.T).astype(bf16)  # [D, T]

    wgT = [np.asarray(gate_proj_w[e], np.float32).T for e in range(E)]
    wuT = [np.asarray(up_proj_w[e], np.float32).T for e in range(E)]
    wdT = [np.asarray(down_proj_w[e], np.float32).T for e in range(E)]

    if _PAIR:
        order = np.argsort(-np.asarray(n_list), kind="stable")
        pairs = [(int(order[k]), int(order[N_CORES - 1 - k]))
                 for k in range(N_CORES // 2)]
        # slot 0 = the SMALLER expert of each pair: its token slot is a
        # single wide chunk, whose slow per-group weight consumption matches
        # the DMA ramp at kernel start
        sA = max(-(-n_list[b] // 2) for _, b in pairs)
        sB = max(-(-n_list[a] // 2) for a, _ in pairs)
        tcsA, tcsB = _slot_chunks(sA), _slot_chunks(sB)
        chunk_plan = tuple([(tc, 0) for tc in tcsA] +
                           [(tc, 1) for tc in tcsB])
        capA = sum(tcsA)
        n_wsets = 2
        core_exp, core_toks = [], []
        for k, (eb, ea) in enumerate(pairs):
            ha = -(-n_list[ea] // 2)
            hb = -(-n_list[eb] // 2)
            for hsel in range(2):
                ta = idx_list[ea][:ha] if hsel == 0 else idx_list[ea][ha:]
                tb = idx_list[eb][:hb] if hsel == 0 else idx_list[eb][hb:]
                core_exp.append((ea, eb))
                core_toks.append((ta, tb))
    else:
        max_n = max(n_list)
        tcs = _slot_chunks(max_n)
        chunk_plan = tuple((tc, 0) for tc in tcs)
        capA = sum(tcs)
        n_wsets = 1
        core_exp = [(c, c) for c in range(N_CORES)]
        core_toks = [(idx_list[c], np.empty(0, np.int64))
                     for c in range(N_CORES)]

    cap = sum(tc for tc, _ in chunk_plan)
    packed_w = {}

    def w_for(e):
        if e not in packed_w:
            packed_w[e] = (_pack_w(wgT[e], bf16, WBLOCKS),
                           _pack_w(wuT[e], bf16, WBLOCKS),
                           _pack_w(wdT[e], bf16))
        return packed_w[e]

    for ea, eb in core_exp:
        w_for(ea)
        if n_wsets > 1:
            w_for(eb)

    def prep_core(c):
        ea, eb = core_exp[c]
        ta, tb = core_toks[c]
        xcT = np.zeros((D, cap), dtype=bf16)
        xcT[:, :len(ta)] = xT_bf[:, ta]
        if len(tb):
            xcT[:, capA:capA + len(tb)] = xT_bf[:, tb]
        # pack x chunk-major: [P, (chunk) do t]
        x3 = xcT.reshape(D_T, P, cap)
        parts, t0 = [], 0
        for tc, _ in chunk_plan:
            parts.append(pack_rows(x3[:, :, t0:t0 + tc]))
            t0 += tc
        xc_p = np.concatenate(parts, axis=1)
        gprob = np.zeros((1, cap), dtype=np.float32)
        gprob[0, :len(ta)] = gating[ta, ea]
        if len(tb):
            gprob[0, capA:capA + len(tb)] = gating[tb, eb]
        wga, wua, wda = w_for(ea)
        if n_wsets > 1:
            wgb, wub, wdb = w_for(eb)
            wg = np.concatenate([wga, wgb], axis=1)
            wu = np.concatenate([wua, wub], axis=1)
            wd = np.concatenate([wda, wdb], axis=1)
        else:
            wg, wu, wd = wga, wua, wda
        return {"xc": xc_p, "wg": wg, "wu": wu, "wd": wd,
                "gprob": gprob.astype(bf16)}

    with ThreadPoolExecutor(N_CORES) as ex:
        in_maps = list(ex.map(prep_core, range(N_CORES)))
    seglists = [(core_toks[c][0], core_toks[c][1], capA)
                for c in range(N_CORES)]
    return in_maps, seglists, chunk_plan, n_wsets


def kernel(x, gate_w, gate_proj_w, up_proj_w, down_proj_w,
           num_experts_per_tok=2, _trace=False, _trace_cores=None):
    from concourse import bass_utils
    assert int(num_experts_per_tok) == TOPK

    kwargs = {}
    if _trace:
        try:
            sys.path.insert(0, os.path.dirname(os.path.abspath(__file__)))
            import axon_profile_shim
            axon_profile_shim.install()
        except Exception as exc:  # profiling is best-effort
            _log(f"profile shim unavailable: {exc}")
        kwargs = dict(trace=True,
                      trace_cores=_trace_cores or list(range(N_CORES)))

    t0 = time.time()
    in_maps, seglists, chunk_plan, n_wsets = make_in_maps_routed(
        x, gate_w, gate_proj_w, up_proj_w, down_proj_w)
    _log(f"host prep {time.time() - t0:.1f}s (chunks={chunk_plan})")
    nc = _get_nc_routed(chunk_plan, n_wsets)
    t0 = time.time()
    res = bass_utils.run_bass_kernel_spmd(
        nc, in_maps, core_ids=list(range(N_CORES)), **kwargs)
    _log(f"run_bass_kernel_spmd took {time.time() - t0:.1f}s")
    kernel.last_result = res
    t0 = time.time()
    y = np.zeros((T, D), dtype=np.float32)
    for c in range(N_CORES):
        yc = np.asarray(res.results[c]["ycomp"], dtype=np.float32)
        ta, tb, capA = seglists[c]
        if len(ta):
            y[ta] += np.ascontiguousarray(yc[:, :len(ta)].T)
        if len(tb):
            y[tb] += np.ascontiguousarray(yc[:, capA:capA + len(tb)].T)
    _log(f"host combine {time.time() - t0:.1f}s")
    return y.reshape(B, S, D)


kernel.last_result = None


# revision 22
# speedup vs baseline: 1.1641x; 1.1641x over previous
"""MoE feed-forward (8 experts, top-2, SwiGLU) on 8 Trainium2 NeuronCores.

Strategy: routed expert parallelism with pair balancing. Experts are paired
largest-count-with-smallest; the two cores of a pair each hold BOTH experts'
weights (bf16 halves them to ~17MB, fitting SBUF) and each processes half of
both experts' routed tokens, so per-core work tracks the pair mean instead
of the max expert. Host gathers each core's tokens compactly (feature-major),
computes the router top-2 softmax exactly, and scatter-adds the compact
per-core outputs into the full [B, S, D] result.

Compute is bf16 (fp32 PSUM accumulate): the PE streams 1 column/cycle for
both fp32r and bf16, but bf16 halves DMA bytes and, via fast-weight-load,
halves LDWEIGHTS so the matmul stream — not the weight loads — is the
limiter. All device inputs are host-packed so every DMA is one contiguous
run per partition (descriptor generation is ~4ns/descriptor on the issuing
engine and would otherwise serialize the head). Inputs stream on two HWDGE
queues (weights on sync, activations on scalar), outputs on the gpsimd
queue so no input queue's end-of-kernel drain waits on them. A few
memset-fed warmup matmuls fill the PE queue during the DMA-limited head.
Measured end-to-end rel-err ~4e-3 (gate: 2e-2).
"""

import os
import sys
import time

sys.path.insert(0, "/opt/trn_rl_repo")

import numpy as np

# ---------------------------------------------------------------------------
# Problem constants (hardcoded per contract)
B, S, D, E, I, TOPK = 2, 2048, 1024, 8, 1408, 2
T = B * S  # 4096 tokens
P = 128
D_T = D // P   # 8 d-tiles
I_T = I // P   # 11 i-tiles
N_CORES = 8
N_WARMUP = 7   # dummy MMs filling the pre-supply window (PE idle anyway)
# i-tile blocks for the wg/wu weight streams (pipelined arrival)
WBLOCKS = [(0, 2), (2, 5), (5, 8), (8, 11)]
WSZ = D_T * I   # one weight set's wg/wu columns per partition
DSZ = I_T * D   # one weight set's wd columns per partition

_VERBOSE = bool(int(os.environ.get("KERNEL_VERBOSE", "0")))
_PAIR = bool(int(os.environ.get("KERNEL_PAIR", "0")))


def _log(msg):
    if _VERBOSE:
        print(f"[kernel] {msg}", flush=True)


def host_gating(x2d: np.ndarray, gate_w: np.ndarray):
    """Exact router: scores -> top-2 -> softmax. Returns gating [T, E] fp32."""
    scores = x2d.astype(np.float64) @ gate_w.astype(np.float64).T  # [T, E]
    idx = np.argsort(-scores, axis=-1, kind="stable")[:, :TOPK]  # [T, 2]
    top = np.take_along_axis(scores, idx, axis=-1)  # [T, 2] descending
    m = top[:, :1]
    ex = np.exp(top - m)
    probs = ex / ex.sum(axis=-1, keepdims=True)  # [T, 2]
    gating = np.zeros((x2d.shape[0], E), dtype=np.float64)
    np.put_along_axis(gating, idx, probs, axis=-1)
    return gating.astype(np.float32)


def pack_rows(a3: np.ndarray) -> np.ndarray:
    """[G, P, W] -> [P, G*W]: per-partition-contiguous device layout."""
    return np.ascontiguousarray(a3.transpose(1, 0, 2).reshape(P, -1))


# ---------------------------------------------------------------------------
# Bass kernel builder


def build_nc_routed(chunk_plan, n_wsets, n_cores=N_CORES):
    """Generalized routed builder. chunk_plan: tuple of (tc, wset) per chunk;
    each core computes SwiGLU for its host-gathered token columns, chunk ci
    using weight set chunk_plan[ci][1], scales by the gating prob, and
    writes the compact [D, cap] output."""
    import concourse.mybir as mybir
    import concourse.tile as tile
    from concourse import bacc

    f32 = mybir.dt.float32
    bf16 = mybir.dt.bfloat16
    cap = sum(tc for tc, _ in chunk_plan)
    tc_max = max(tc for tc, _ in chunk_plan)
    n_chunks = len(chunk_plan)

    nc = bacc.Bacc("TRN2", debug=False, num_devices=n_cores)

    xc_d = nc.dram_tensor("xc", [P, D_T * cap], bf16, kind="ExternalInput")
    wg_d = nc.dram_tensor("wg", [P, n_wsets * WSZ], bf16, kind="ExternalInput")
    wu_d = nc.dram_tensor("wu", [P, n_wsets * WSZ], bf16, kind="ExternalInput")
    wd_d = nc.dram_tensor("wd", [P, n_wsets * DSZ], bf16, kind="ExternalInput")
    gprob_d = nc.dram_tensor("gprob", [1, cap], bf16, kind="ExternalInput")
    ycomp_d = nc.dram_tensor("ycomp", [D, cap], bf16, kind="ExternalOutput")
    ycomp_r = ycomp_d.ap().rearrange("(do dp) t -> dp do t", dp=P)

    # flat offsets for the block-major wg/wu layout
    blk_of_tile = {}
    blk_off = {}
    off = 0
    for b, (s0, s1) in enumerate(WBLOCKS):
        blk_off[b] = off
        for i_o in range(s0, s1):
            blk_of_tile[i_o] = (b, i_o - s0)
        off += D_T * (s1 - s0) * P
    assert off == WSZ

    def w_slice(w_sb, ws, d_o, i_o):
        b, j = blk_of_tile[i_o]
        s0, s1 = WBLOCKS[b]
        bw = (s1 - s0) * P
        o = ws * WSZ + blk_off[b] + d_o * bw + j * P
        return w_sb[:, o:o + P]

    def blk_rng(b):
        s0, s1 = WBLOCKS[b]
        return blk_off[b], blk_off[b] + D_T * (s1 - s0) * P

    # chunk column offsets
    xoff, toff = [], []
    xo = to = 0
    for tc, _ in chunk_plan:
        xoff.append(xo)
        toff.append(to)
        xo += D_T * tc
        to += tc

    with tile.TileContext(nc) as tcx:
        with tcx.tile_pool(name="wpool", bufs=1) as wpool, \
             tcx.tile_pool(name="xpool", bufs=n_chunks) as xpool, \
             tcx.tile_pool(name="hpool", bufs=2) as hpool, \
             tcx.tile_pool(name="ypool", bufs=2) as ypool, \
             tcx.tile_pool(name="gspool", bufs=3) as gspool, \
             tcx.tile_pool(name="gbpool", bufs=2) as gbpool, \
             tcx.tile_pool(name="psg", bufs=2, space="PSUM") as psg, \
             tcx.tile_pool(name="psu", bufs=2, space="PSUM") as psu, \
             tcx.tile_pool(name="psy", bufs=2, space="PSUM") as psy, \
             tcx.tile_pool(name="psb", bufs=1, space="PSUM") as psb:

            # ---- consts: memset, no DMA ----
            warm_c = wpool.tile([1, 512], bf16)
            nc.gpsimd.memset(warm_c[:], 1.0)

            # PE queue filler for the DMA-limited head
            warm_ps = psb.tile([P, 512], f32, tag="gbps", name="warm")
            for _ in range(N_WARMUP):
                nc.tensor.matmul(warm_ps[:], warm_c[:, :P], warm_c[:],
                                 start=True, stop=True)

            wg_sb = wpool.tile([P, n_wsets * WSZ], bf16)
            wu_sb = wpool.tile([P, n_wsets * WSZ], bf16)
            wd_sb = wpool.tile([P, n_wsets * DSZ], bf16)
            gprob_sb = wpool.tile([1, cap], bf16)

            xts = [xpool.tile([P, D_T * tc], bf16, tag="xt", name=f"xt{ci}")
                   for ci, (tc, _) in enumerate(chunk_plan)]

            # activations on the scalar HWDGE queue; weights stream on the
            # sync queue in consumption order (wg/wu interleaved by block)
            tc0 = chunk_plan[0][0]
            for s in range(D_T):  # chunk 0 per-d_o: earliest first matmul
                nc.scalar.dma_start(xts[0][:, s * tc0:(s + 1) * tc0],
                                    xc_d.ap()[:, s * tc0:(s + 1) * tc0])
            nc.scalar.dma_start(gprob_sb[:], gprob_d.ap())
            for ci in range(1, n_chunks):
                o = xoff[ci]
                nc.scalar.dma_start(
                    xts[ci][:], xc_d.ap()[:, o:o + D_T * chunk_plan[ci][0]])

            for b in range(len(WBLOCKS)):
                o0, o1 = blk_rng(b)
                nc.sync.dma_start(wg_sb[:, o0:o1], wg_d.ap()[:, o0:o1])
                nc.sync.dma_start(wu_sb[:, o0:o1], wu_d.ap()[:, o0:o1])
            nc.sync.dma_start(wd_sb[:, :DSZ], wd_d.ap()[:, :DSZ])
            if n_wsets > 1:  # weight set 1 streams behind set 0's compute
                nc.sync.dma_start(wg_sb[:, WSZ:], wg_d.ap()[:, WSZ:])
                nc.sync.dma_start(wu_sb[:, WSZ:], wu_d.ap()[:, WSZ:])
                nc.sync.dma_start(wd_sb[:, DSZ:], wd_d.ap()[:, DSZ:])

            for ci, (tc, ws) in enumerate(chunk_plan):
                t0 = toff[ci]
                xt = xts[ci]

                h = hpool.tile([P, I_T, tc_max], bf16, tag="h")
                for i_o in range(I_T):
                    pg = psg.tile([P, tc_max], f32, tag="pg")
                    pu = psu.tile([P, tc_max], f32, tag="pu")
                    for d_o in range(D_T):
                        nc.tensor.matmul(
                            pg[:, :tc], w_slice(wg_sb, ws, d_o, i_o),
                            xt[:, d_o * tc:(d_o + 1) * tc],
                            start=(d_o == 0), stop=(d_o == D_T - 1))
                    for d_o in range(D_T):
                        nc.tensor.matmul(
                            pu[:, :tc], w_slice(wu_sb, ws, d_o, i_o),
                            xt[:, d_o * tc:(d_o + 1) * tc],
                            start=(d_o == 0), stop=(d_o == D_T - 1))
                    gs = gspool.tile([P, tc_max], bf16, tag="gs")
                    nc.scalar.activation(gs[:, :tc], pg[:, :tc],
                                         mybir.ActivationFunctionType.Silu)
                    nc.vector.tensor_mul(out=h[:, i_o, :tc], in0=gs[:, :tc],
                                         in1=pu[:, :tc])

                # broadcast this chunk's gating row to 128 partitions just
                # before the down-proj that consumes it
                gb_ps = psb.tile([P, tc_max], f32, tag="gbps")
                nc.tensor.matmul(gb_ps[:, :tc], warm_c[:, :P],
                                 gprob_sb[:, t0:t0 + tc],
                                 start=True, stop=True)
                gb_sb = gbpool.tile([P, tc_max], f32, tag="gb")
                nc.vector.tensor_copy(out=gb_sb[:, :tc], in_=gb_ps[:, :tc])

                yout = ypool.tile([P, D_T, tc_max], bf16, tag="yout")
                for d_o in range(D_T):
                    py = psy.tile([P, tc_max], f32, tag="py")
                    for i_o in range(I_T):
                        nc.tensor.matmul(
                            py[:, :tc],
                            wd_sb[:, ws * DSZ + i_o * D + d_o * P:
                                  ws * DSZ + i_o * D + d_o * P + P],
                            h[:, i_o, :tc],
                            start=(i_o == 0), stop=(i_o == I_T - 1))
                    nc.vector.tensor_mul(out=yout[:, d_o, :tc],
                                         in0=py[:, :tc], in1=gb_sb[:, :tc])
                # outputs on the (otherwise idle) gpsimd queue; the LAST
                # chunk's go per-d_o on the scalar queue instead — HWDGE
                # descriptor gen is ~3x faster than gpsimd's SWDGE, and the
                # scalar engine is idle during the final down-proj, so the
                # post-last-matmul drain shrinks
                if ci == n_chunks - 1:
                    for s0 in range(D_T):
                        nc.scalar.dma_start(
                            ycomp_r[:, s0:s0 + 1, t0:t0 + tc],
                            yout[:, s0:s0 + 1, :tc])
                else:
                    for s0 in range(0, D_T, 2):
                        nc.gpsimd.dma_start(
                            ycomp_r[:, s0:s0 + 2, t0:t0 + tc],
                            yout[:, s0:s0 + 2, :tc])

    nc.compile()
    return nc


# ---------------------------------------------------------------------------
# Host-side wrapper

_CACHED = {}


def _get_nc_routed(chunk_plan, n_wsets, n_cores=N_CORES):
    key = (chunk_plan, n_wsets, n_cores)
    if key not in _CACHED:
        t0 = time.time()
        _CACHED[key] = build_nc_routed(chunk_plan, n_wsets, n_cores)
        _log(f"built bass program (chunks={chunk_plan}, wsets={n_wsets}) "
             f"in {time.time() - t0:.1f}s")
    return _CACHED[key]


def _round_up(v, m):
    return (v + m - 1) // m * m


def _slot_chunks(s):
    """Chunk widths covering s columns: fewest <=512 chunks, evenly sized."""
    n = max(1, -(-s // 512))
    tc = _round_up(-(-s // n), 4)
    return [tc] * n


def _pack_w(w, bf16, blocks=None):
    """w: [G*P, W] source -> device flat [P, G*W] (optionally block-major)."""
    a3 = w.reshape(-1, P, w.shape[-1])  # [G, P, W]
    if blocks is None:
        return pack_rows(a3).astype(bf16)
    parts = [pack_rows(a3[:, :, s0 * P:s1 * P]) for s0, s1 in blocks]
    return np.concatenate(parts, axis=1).astype(bf16)


def make_in_maps_routed(x, gate_w, gate_proj_w, up_proj_w, down_proj_w):
    """Returns (in_maps, seglists, chunk_plan, n_wsets)."""
    from concurrent.futures import ThreadPoolExecutor
    import ml_dtypes

    bf16 = ml_dtypes.bfloat16
    x2d = np.ascontiguousarray(np.asarray(x, np.float32).reshape(T, D))
    gating = host_gating(x2d, np.asarray(gate_w, np.float32))  # [T, E]
    idx_list = [np.nonzero(gating[:, c] > 0)[0].astype(np.int64)
                for c in range(N_CORES)]
    n_list = [len(ix) for ix in idx_list]
    xT_bf = np.ascontiguousarray(x2d.T).astype(bf16)  # [D, T]

    wgT = [np.asarray(gate_proj_w[e], np.float32).T for e in range(E)]
    wuT = [np.asarray(up_proj_w[e], np.float32).T for e in range(E)]
    wdT = [np.asarray(down_proj_w[e], np.float32).T for e in range(E)]

    if _PAIR:
        order = np.argsort(-np.asarray(n_list), kind="stable")
        pairs = [(int(order[k]), int(order[N_CORES - 1 - k]))
                 for k in range(N_CORES // 2)]
        # slot 0 = the SMALLER expert of each pair: its token slot is a
        # single wide chunk, whose slow per-group weight consumption matches
        # the DMA ramp at kernel start
        sA = max(-(-n_list[b] // 2) for _, b in pairs)
        sB = max(-(-n_list[a] // 2) for a, _ in pairs)
        tcsA, tcsB = _slot_chunks(sA), _slot_chunks(sB)
        chunk_plan = tuple([(tc, 0) for tc in tcsA] +
                           [(tc, 1) for tc in tcsB])
        capA = sum(tcsA)
        n_wsets = 2
        core_exp, core_toks = [], []
        for k, (eb, ea) in enumerate(pairs):
            ha = -(-n_list[ea] // 2)
            hb = -(-n_list[eb] // 2)
            for hsel in range(2):
                ta = idx_list[ea][:ha] if hsel == 0 else idx_list[ea][ha:]
                tb = idx_list[eb][:hb] if hsel == 0 else idx_list[eb][hb:]
                core_exp.append((ea, eb))
                core_toks.append((ta, tb))
    else:
        max_n = max(n_list)
        tcs = _slot_chunks(max_n)
        chunk_plan = tuple((tc, 0) for tc in tcs)
        capA = sum(tcs)
        n_wsets = 1
        core_exp = [(c, c) for c in range(N_CORES)]
        core_toks = [(idx_list[c], np.empty(0, np.int64))
                     for c in range(N_CORES)]

    cap = sum(tc for tc, _ in chunk_plan)
    packed_w = {}

    def w_for(e):
        if e not in packed_w:
            packed_w[e] = (_pack_w(wgT[e], bf16, WBLOCKS),
                           _pack_w(wuT[e], bf16, WBLOCKS),
                           _pack_w(wdT[e], bf16))
        return packed_w[e]

    for ea, eb in core_exp:
        w_for(ea)
        if n_wsets > 1:
            w_for(eb)

    def prep_core(c):
        ea, eb = core_exp[c]
        ta, tb = core_toks[c]
        xcT = np.zeros((D, cap), dtype=bf16)
        xcT[:, :len(ta)] = xT_bf[:, ta]
        if len(tb):
            xcT[:, capA:capA + len(tb)] = xT_bf[:, tb]
        # pack x chunk-major: [P, (chunk) do t]
        x3 = xcT.reshape(D_T, P, cap)
        parts, t0 = [], 0
        for tc, _ in chunk_plan:
            parts.append(pack_rows(x3[:, :, t0:t0 + tc]))
            t0 += tc
        xc_p = np.concatenate(parts, axis=1)
        gprob = np.zeros((1, cap), dtype=np.float32)
        gprob[0, :len(ta)] = gating[ta, ea]
        if len(tb):
            gprob[0, capA:capA + len(tb)] = gating[tb, eb]
        wga, wua, wda = w_for(ea)
        if n_wsets > 1:
            wgb, wub, wdb = w_for(eb)
            wg = np.concatenate([wga, wgb], axis=1)
            wu = np.concatenate([wua, wub], axis=1)
            wd = np.concatenate([wda, wdb], axis=1)
        else:
            wg, wu, wd = wga, wua, wda
        return {"xc": xc_p, "wg": wg, "wu": wu, "wd": wd,
                "gprob": gprob.astype(bf16)}

    with ThreadPoolExecutor(N_CORES) as ex:
        in_maps = list(ex.map(prep_core, range(N_CORES)))
    seglists = [(core_toks[c][0], core_toks[c][1], capA)
                for c in range(N_CORES)]
    return in_maps, seglists, chunk_plan, n_wsets


def kernel(x, gate_w, gate_proj_w, up_proj_w, down_proj_w,
           num_experts_per_tok=2, _trace=False, _trace_cores=None):
    from concourse import bass_utils
    assert int(num_experts_per_tok) == TOPK

    kwargs = {}
    if _trace:
        try:
            sys.path.insert(0, os.path.dirname(os.path.abspath(__file__)))
            import axon_profile_shim
            axon_profile_shim.install()
        except Exception as exc:  # profiling is best-effort
            _log(f"profile shim unavailable: {exc}")
        kwargs = dict(trace=True,
                      trace_cores=_trace_cores or list(range(N_CORES)))

    t0 = time.time()
    in_maps, seglists, chunk_plan, n_wsets = make_in_maps_routed(
        x, gate_w, gate_proj_w, up_proj_w, down_proj_w)
    _log(f"host prep {time.time() - t0:.1f}s (chunks={chunk_plan})")
    nc = _get_nc_routed(chunk_plan, n_wsets)
    t0 = time.time()
    res = bass_utils.run_bass_kernel_spmd(
        nc, in_maps, core_ids=list(range(N_CORES)), **kwargs)
    _log(f"run_bass_kernel_spmd took {time.time() - t0:.1f}s")
    kernel.last_result = res
    t0 = time.time()
    y = np.zeros((T, D), dtype=np.float32)
    for c in range(N_CORES):
        yc = np.asarray(res.results[c]["ycomp"], dtype=np.float32)
        ta, tb, capA = seglists[c]
        if len(ta):
            y[ta] += np.ascontiguousarray(yc[:, :len(ta)].T)
        if len(tb):
            y[tb] += np.ascontiguousarray(yc[:, capA:capA + len(tb)].T)
    _log(f"host combine {time.time() - t0:.1f}s")
    return y.reshape(B, S, D)


kernel.last_result = None


# revision 23
# speedup vs baseline: 1.1694x; 1.0045x over previous
"""MoE feed-forward (8 experts, top-2, SwiGLU) on 8 Trainium2 NeuronCores.

Strategy: routed expert parallelism with pair balancing. Experts are paired
largest-count-with-smallest; the two cores of a pair each hold BOTH experts'
weights (bf16 halves them to ~17MB, fitting SBUF) and each processes half of
both experts' routed tokens, so per-core work tracks the pair mean instead
of the max expert. Host gathers each core's tokens compactly (feature-major),
computes the router top-2 softmax exactly, and scatter-adds the compact
per-core outputs into the full [B, S, D] result.

Compute is bf16 (fp32 PSUM accumulate): the PE streams 1 column/cycle for
both fp32r and bf16, but bf16 halves DMA bytes and, via fast-weight-load,
halves LDWEIGHTS so the matmul stream — not the weight loads — is the
limiter. All device inputs are host-packed so every DMA is one contiguous
run per partition (descriptor generation is ~4ns/descriptor on the issuing
engine and would otherwise serialize the head). Inputs stream on two HWDGE
queues (weights on sync, activations on scalar), outputs on the gpsimd
queue so no input queue's end-of-kernel drain waits on them. A few
memset-fed warmup matmuls fill the PE queue during the DMA-limited head.
Measured end-to-end rel-err ~4e-3 (gate: 2e-2).
"""

import os
import sys
import time

sys.path.insert(0, "/opt/trn_rl_repo")

import numpy as np

# ---------------------------------------------------------------------------
# Problem constants (hardcoded per contract)
B, S, D, E, I, TOPK = 2, 2048, 1024, 8, 1408, 2
T = B * S  # 4096 tokens
P = 128
D_T = D // P   # 8 d-tiles
I_T = I // P   # 11 i-tiles
N_CORES = 8
N_WARMUP = 7   # dummy MMs filling the pre-supply window (PE idle anyway)
# i-tile blocks for the wg/wu weight streams (pipelined arrival)
WBLOCKS = [(0, 2), (2, 5), (5, 8), (8, 11)]
WSZ = D_T * I   # one weight set's wg/wu columns per partition
DSZ = I_T * D   # one weight set's wd columns per partition

_VERBOSE = bool(int(os.environ.get("KERNEL_VERBOSE", "0")))
_PAIR = bool(int(os.environ.get("KERNEL_PAIR", "0")))


def _log(msg):
    if _VERBOSE:
        print(f"[kernel] {msg}", flush=True)


def host_gating(x2d: np.ndarray, gate_w: np.ndarray):
    """Exact router: scores -> top-2 -> softmax. Returns gating [T, E] fp32."""
    scores = x2d.astype(np.float64) @ gate_w.astype(np.float64).T  # [T, E]
    idx = np.argsort(-scores, axis=-1, kind="stable")[:, :TOPK]  # [T, 2]
    top = np.take_along_axis(scores, idx, axis=-1)  # [T, 2] descending
    m = top[:, :1]
    ex = np.exp(top - m)
    probs = ex / ex.sum(axis=-1, keepdims=True)  # [T, 2]
    gating = np.zeros((x2d.shape[0], E), dtype=np.float64)
    np.put_along_axis(gating, idx, probs, axis=-1)
    return gating.astype(np.float32)


def pack_rows(a3: np.ndarray) -> np.ndarray:
    """[G, P, W] -> [P, G*W]: per-partition-contiguous device layout."""
    return np.ascontiguousarray(a3.transpose(1, 0, 2).reshape(P, -1))


# ---------------------------------------------------------------------------
# Bass kernel builder


def build_nc_routed(chunk_plan, n_wsets, n_cores=N_CORES):
    """Generalized routed builder. chunk_plan: tuple of (tc, wset) per chunk;
    each core computes SwiGLU for its host-gathered token columns, chunk ci
    using weight set chunk_plan[ci][1], scales by the gating prob, and
    writes the compact [D, cap] output."""
    import concourse.mybir as mybir
    import concourse.tile as tile
    from concourse import bacc

    f32 = mybir.dt.float32
    bf16 = mybir.dt.bfloat16
    cap = sum(tc for tc, _ in chunk_plan)
    tc_max = max(tc for tc, _ in chunk_plan)
    n_chunks = len(chunk_plan)

    nc = bacc.Bacc("TRN2", debug=False, num_devices=n_cores)

    xc_d = nc.dram_tensor("xc", [P, D_T * cap], bf16, kind="ExternalInput")
    wg_d = nc.dram_tensor("wg", [P, n_wsets * WSZ], bf16, kind="ExternalInput")
    wu_d = nc.dram_tensor("wu", [P, n_wsets * WSZ], bf16, kind="ExternalInput")
    wd_d = nc.dram_tensor("wd", [P, n_wsets * DSZ], bf16, kind="ExternalInput")
    gprob_d = nc.dram_tensor("gprob", [1, cap], bf16, kind="ExternalInput")
    ycomp_d = nc.dram_tensor("ycomp", [D, cap], bf16, kind="ExternalOutput")
    ycomp_r = ycomp_d.ap().rearrange("(do dp) t -> dp do t", dp=P)

    # flat offsets for the block-major wg/wu layout
    blk_of_tile = {}
    blk_off = {}
    off = 0
    for b, (s0, s1) in enumerate(WBLOCKS):
        blk_off[b] = off
        for i_o in range(s0, s1):
            blk_of_tile[i_o] = (b, i_o - s0)
        off += D_T * (s1 - s0) * P
    assert off == WSZ

    def w_slice(w_sb, ws, d_o, i_o):
        b, j = blk_of_tile[i_o]
        s0, s1 = WBLOCKS[b]
        bw = (s1 - s0) * P
        o = ws * WSZ + blk_off[b] + d_o * bw + j * P
        return w_sb[:, o:o + P]

    def blk_rng(b):
        s0, s1 = WBLOCKS[b]
        return blk_off[b], blk_off[b] + D_T * (s1 - s0) * P

    # chunk column offsets
    xoff, toff = [], []
    xo = to = 0
    for tc, _ in chunk_plan:
        xoff.append(xo)
        toff.append(to)
        xo += D_T * tc
        to += tc

    with tile.TileContext(nc) as tcx:
        with tcx.tile_pool(name="wpool", bufs=1) as wpool, \
             tcx.tile_pool(name="xpool", bufs=n_chunks) as xpool, \
             tcx.tile_pool(name="hpool", bufs=2) as hpool, \
             tcx.tile_pool(name="ypool", bufs=2) as ypool, \
             tcx.tile_pool(name="gspool", bufs=3) as gspool, \
             tcx.tile_pool(name="gbpool", bufs=2) as gbpool, \
             tcx.tile_pool(name="psg", bufs=2, space="PSUM") as psg, \
             tcx.tile_pool(name="psu", bufs=2, space="PSUM") as psu, \
             tcx.tile_pool(name="psy", bufs=2, space="PSUM") as psy, \
             tcx.tile_pool(name="psb", bufs=1, space="PSUM") as psb:

            # ---- consts: memset, no DMA ----
            warm_c = wpool.tile([1, 512], bf16)
            nc.gpsimd.memset(warm_c[:], 1.0)

            # PE queue filler for the DMA-limited head
            warm_ps = psb.tile([P, 512], f32, tag="gbps", name="warm")
            for _ in range(N_WARMUP):
                nc.tensor.matmul(warm_ps[:], warm_c[:, :P], warm_c[:],
                                 start=True, stop=True)

            wg_sb = wpool.tile([P, n_wsets * WSZ], bf16)
            wu_sb = wpool.tile([P, n_wsets * WSZ], bf16)
            wd_sb = wpool.tile([P, n_wsets * DSZ], bf16)
            gprob_sb = wpool.tile([1, cap], bf16)

            xts = [xpool.tile([P, D_T * tc], bf16, tag="xt", name=f"xt{ci}")
                   for ci, (tc, _) in enumerate(chunk_plan)]

            # activations on the scalar HWDGE queue; weights stream on the
            # sync queue in consumption order (wg/wu interleaved by block)
            tc0 = chunk_plan[0][0]
            for s in range(0, D_T, 2):  # chunk 0 in d_o pairs
                nc.scalar.dma_start(xts[0][:, s * tc0:(s + 2) * tc0],
                                    xc_d.ap()[:, s * tc0:(s + 2) * tc0])
            nc.scalar.dma_start(gprob_sb[:], gprob_d.ap())
            for ci in range(1, n_chunks):
                o = xoff[ci]
                nc.scalar.dma_start(
                    xts[ci][:], xc_d.ap()[:, o:o + D_T * chunk_plan[ci][0]])

            for b in range(len(WBLOCKS)):
                o0, o1 = blk_rng(b)
                nc.sync.dma_start(wg_sb[:, o0:o1], wg_d.ap()[:, o0:o1])
                nc.sync.dma_start(wu_sb[:, o0:o1], wu_d.ap()[:, o0:o1])
            nc.sync.dma_start(wd_sb[:, :DSZ], wd_d.ap()[:, :DSZ])
            if n_wsets > 1:  # weight set 1 streams behind set 0's compute
                nc.sync.dma_start(wg_sb[:, WSZ:], wg_d.ap()[:, WSZ:])
                nc.sync.dma_start(wu_sb[:, WSZ:], wu_d.ap()[:, WSZ:])
                nc.sync.dma_start(wd_sb[:, DSZ:], wd_d.ap()[:, DSZ:])

            for ci, (tc, ws) in enumerate(chunk_plan):
                t0 = toff[ci]
                xt = xts[ci]

                h = hpool.tile([P, I_T, tc_max], bf16, tag="h")
                for i_o in range(I_T):
                    pg = psg.tile([P, tc_max], f32, tag="pg")
                    pu = psu.tile([P, tc_max], f32, tag="pu")
                    for d_o in range(D_T):
                        nc.tensor.matmul(
                            pg[:, :tc], w_slice(wg_sb, ws, d_o, i_o),
                            xt[:, d_o * tc:(d_o + 1) * tc],
                            start=(d_o == 0), stop=(d_o == D_T - 1))
                    for d_o in range(D_T):
                        nc.tensor.matmul(
                            pu[:, :tc], w_slice(wu_sb, ws, d_o, i_o),
                            xt[:, d_o * tc:(d_o + 1) * tc],
                            start=(d_o == 0), stop=(d_o == D_T - 1))
                    gs = gspool.tile([P, tc_max], bf16, tag="gs")
                    nc.scalar.activation(gs[:, :tc], pg[:, :tc],
                                         mybir.ActivationFunctionType.Silu)
                    nc.vector.tensor_mul(out=h[:, i_o, :tc], in0=gs[:, :tc],
                                         in1=pu[:, :tc])

                # broadcast this chunk's gating row to 128 partitions just
                # before the down-proj that consumes it
                gb_ps = psb.tile([P, tc_max], f32, tag="gbps")
                nc.tensor.matmul(gb_ps[:, :tc], warm_c[:, :P],
                                 gprob_sb[:, t0:t0 + tc],
                                 start=True, stop=True)
                gb_sb = gbpool.tile([P, tc_max], f32, tag="gb")
                nc.vector.tensor_copy(out=gb_sb[:, :tc], in_=gb_ps[:, :tc])

                yout = ypool.tile([P, D_T, tc_max], bf16, tag="yout")
                for d_o in range(D_T):
                    py = psy.tile([P, tc_max], f32, tag="py")
                    for i_o in range(I_T):
                        nc.tensor.matmul(
                            py[:, :tc],
                            wd_sb[:, ws * DSZ + i_o * D + d_o * P:
                                  ws * DSZ + i_o * D + d_o * P + P],
                            h[:, i_o, :tc],
                            start=(i_o == 0), stop=(i_o == I_T - 1))
                    nc.vector.tensor_mul(out=yout[:, d_o, :tc],
                                         in0=py[:, :tc], in1=gb_sb[:, :tc])
                # outputs on the (otherwise idle) gpsimd queue; last chunk
                # per-d_o so the final transfer after the last matmul is small
                ostep = 1 if ci == n_chunks - 1 else 2
                for s0 in range(0, D_T, ostep):
                    nc.gpsimd.dma_start(
                        ycomp_r[:, s0:s0 + ostep, t0:t0 + tc],
                        yout[:, s0:s0 + ostep, :tc])

    nc.compile()
    return nc


# ---------------------------------------------------------------------------
# Host-side wrapper

_CACHED = {}


def _get_nc_routed(chunk_plan, n_wsets, n_cores=N_CORES):
    key = (chunk_plan, n_wsets, n_cores)
    if key not in _CACHED:
        t0 = time.time()
        _CACHED[key] = build_nc_routed(chunk_plan, n_wsets, n_cores)
        _log(f"built bass program (chunks={chunk_plan}, wsets={n_wsets}) "
             f"in {time.time() - t0:.1f}s")
    return _CACHED[key]


def _round_up(v, m):
    return (v + m - 1) // m * m


def _slot_chunks(s):
    """Chunk widths covering s columns: fewest <=512 chunks, evenly sized."""
    n = max(1, -(-s // 512))
    tc = _round_up(-(-s // n), 4)
    return [tc] * n


def _pack_w(w, bf16, blocks=None):
    """w: [G*P, W] source -> device flat [P, G*W] (optionally block-major)."""
    a3 = w.reshape(-1, P, w.shape[-1])  # [G, P, W]
    if blocks is None:
        return pack_rows(a3).astype(bf16)
    parts = [pack_rows(a3[:, :, s0 * P:s1 * P]) for s0, s1 in blocks]
    return np.concatenate(parts, axis=1).astype(bf16)


def make_in_maps_routed(x, gate_w, gate_proj_w, up_proj_w, down_proj_w):
    """Returns (in_maps, seglists, chunk_plan, n_wsets)."""
    from concurrent.futures import ThreadPoolExecutor
    import ml_dtypes

    bf16 = ml_dtypes.bfloat16
    x2d = np.ascontiguousarray(np.asarray(x, np.float32).reshape(T, D))
    gating = host_gating(x2d, np.asarray(gate_w, np.float32))  # [T, E]
    idx_list = [np.nonzero(gating[:, c] > 0)[0].astype(np.int64)
                for c in range(N_CORES)]
    n_list = [len(ix) for ix in idx_list]
    xT_bf = np.ascontiguousarray(x2d.T).astype(bf16)  # [D, T]

    wgT = [np.asarray(gate_proj_w[e], np.float32).T for e in range(E)]
    wuT = [np.asarray(up_proj_w[e], np.float32).T for e in range(E)]
    wdT = [np.asarray(down_proj_w[e], np.float32).T for e in range(E)]

    if _PAIR:
        order = np.argsort(-np.asarray(n_list), kind="stable")
        pairs = [(int(order[k]), int(order[N_CORES - 1 - k]))
                 for k in range(N_CORES // 2)]
        # slot 0 = the SMALLER expert of each pair: its token slot is a
        # single wide chunk, whose slow per-group weight consumption matches
        # the DMA ramp at kernel start
        sA = max(-(-n_list[b] // 2) for _, b in pairs)
        sB = max(-(-n_list[a] // 2) for a, _ in pairs)
        tcsA, tcsB = _slot_chunks(sA), _slot_chunks(sB)
        chunk_plan = tuple([(tc, 0) for tc in tcsA] +
                           [(tc, 1) for tc in tcsB])
        capA = sum(tcsA)
        n_wsets = 2
        core_exp, core_toks = [], []
        for k, (eb, ea) in enumerate(pairs):
            ha = -(-n_list[ea] // 2)
            hb = -(-n_list[eb] // 2)
            for hsel in range(2):
                ta = idx_list[ea][:ha] if hsel == 0 else idx_list[ea][ha:]
                tb = idx_list[eb][:hb] if hsel == 0 else idx_list[eb][hb:]
                core_exp.append((ea, eb))
                core_toks.append((ta, tb))
    else:
        max_n = max(n_list)
        tcs = _slot_chunks(max_n)
        chunk_plan = tuple((tc, 0) for tc in tcs)
        capA = sum(tcs)
        n_wsets = 1
        core_exp = [(c, c) for c in range(N_CORES)]
        core_toks = [(idx_list[c], np.empty(0, np.int64))
                     for c in range(N_CORES)]

    cap = sum(tc for tc, _ in chunk_plan)
    packed_w = {}

    def w_for(e):
        if e not in packed_w:
            packed_w[e] = (_pack_w(wgT[e], bf16, WBLOCKS),
                           _pack_w(wuT[e], bf16, WBLOCKS),
                           _pack_w(wdT[e], bf16))
        return packed_w[e]

    for ea, eb in core_exp:
        w_for(ea)
        if n_wsets > 1:
            w_for(eb)

    def prep_core(c):
        ea, eb = core_exp[c]
        ta, tb = core_toks[c]
        xcT = np.zeros((D, cap), dtype=bf16)
        xcT[:, :len(ta)] = xT_bf[:, ta]
        if len(tb):
            xcT[:, capA:capA + len(tb)] = xT_bf[:, tb]
        # pack x chunk-major: [P, (chunk) do t]
        x3 = xcT.reshape(D_T, P, cap)
        parts, t0 = [], 0
        for tc, _ in chunk_plan:
            parts.append(pack_rows(x3[:, :, t0:t0 + tc]))
            t0 += tc
        xc_p = np.concatenate(parts, axis=1)
        gprob = np.zeros((1, cap), dtype=np.float32)
        gprob[0, :len(ta)] = gating[ta, ea]
        if len(tb):
            gprob[0, capA:capA + len(tb)] = gating[tb, eb]
        wga, wua, wda = w_for(ea)
        if n_wsets > 1:
            wgb, wub, wdb = w_for(eb)
            wg = np.concatenate([wga, wgb], axis=1)
            wu = np.concatenate([wua, wub], axis=1)
            wd = np.concatenate([wda, wdb], axis=1)
        else:
            wg, wu, wd = wga, wua, wda
        return {"xc": xc_p, "wg": wg, "wu": wu, "wd": wd,
                "gprob": gprob.astype(bf16)}

    with ThreadPoolExecutor(N_CORES) as ex:
        in_maps = list(ex.map(prep_core, range(N_CORES)))
    seglists = [(core_toks[c][0], core_toks[c][1], capA)
                for c in range(N_CORES)]
    return in_maps, seglists, chunk_plan, n_wsets


def kernel(x, gate_w, gate_proj_w, up_proj_w, down_proj_w,
           num_experts_per_tok=2, _trace=False, _trace_cores=None):
    from concourse import bass_utils
    assert int(num_experts_per_tok) == TOPK

    kwargs = {}
    if _trace:
        try:
            sys.path.insert(0, os.path.dirname(os.path.abspath(__file__)))
            import axon_profile_shim
            axon_profile_shim.install()
        except Exception as exc:  # profiling is best-effort
            _log(f"profile shim unavailable: {exc}")
        kwargs = dict(trace=True,
                      trace_cores=_trace_cores or list(range(N_CORES)))

    t0 = time.time()
    in_maps, seglists, chunk_plan, n_wsets = make_in_maps_routed(
        x, gate_w, gate_proj_w, up_proj_w, down_proj_w)
    _log(f"host prep {time.time() - t0:.1f}s (chunks={chunk_plan})")
    nc = _get_nc_routed(chunk_plan, n_wsets)
    t0 = time.time()
    res = bass_utils.run_bass_kernel_spmd(
        nc, in_maps, core_ids=list(range(N_CORES)), **kwargs)
    _log(f"run_bass_kernel_spmd took {time.time() - t0:.1f}s")
    kernel.last_result = res
    t0 = time.time()
    y = np.zeros((T, D), dtype=np.float32)
    for c in range(N_CORES):
        yc = np.asarray(res.results[c]["ycomp"], dtype=np.float32)
        ta, tb, capA = seglists[c]
        if len(ta):
            y[ta] += np.ascontiguousarray(yc[:, :len(ta)].T)
        if len(tb):
            y[tb] += np.ascontiguousarray(yc[:, capA:capA + len(tb)].T)
    _log(f"host combine {time.time() - t0:.1f}s")
    return y.reshape(B, S, D)


kernel.last_result = None
